# revision 1
# baseline (speedup 1.0000x reference)
# Trainium2 Bass kernel for nn_Action2 (invest-consumption SDE scan with two
# small MLPs per step). Data-parallel across 8 NeuronCores: batch 8192 -> 1024
# per core, split into TWO independent 512-wide streams whose serial update
# chains interleave in anti-phase. All-fp16 feature/weight rows (fp32 PSUM
# accumulation) collapse layer 1 into a single K=46 matmul per stream-step
# with the layer-1 bias folded in as constant-one / t feature rows; layer 3
# is a single M=2 matmul and the update domain is [32, 16]. Crossings are one
# 32x32-blockwise DVE transpose each way; update math runs in-order on Vector
# with exp on Scalar in parallel; relus run on Scalar/Pool/Vector splits.
import os
import sys

import numpy as np

for _p in ("/opt/trn_rl_repo",):
    if _p not in sys.path:
        sys.path.insert(0, _p)

import concourse.bacc as bacc  # noqa: E402
import concourse.mybir as mybir  # noqa: E402
import concourse.tile as tile  # noqa: E402
from concourse.bass_utils import run_bass_kernel_spmd  # noqa: E402
from concourse.tile_rust import add_dep_helper  # noqa: E402

F32 = mybir.dt.float32
F16 = mybir.dt.float16
ALU = mybir.AluOpType
ACTF = mybir.ActivationFunctionType

B_GLOBAL = 8192
N_CORES = 8
B = B_GLOBAL // N_CORES   # 1024 per core
SB = B // 2               # 512 per stream
N_STEPS = 100
IN_DIM = 5
T_HORIZON = 1.0
MU, NU, SIGMA = 0.1, 0.2, 0.3
BN_EPS = 1e-5
DT = T_HORIZON / N_STEPS

# Per-stream batch index: bs = 256*cl + 32*fb + q.  Update domain [32, 16]:
# partition q, col c = 8*cl + fb (so bs = 32*c + q).  128-col chunk
# j = bs//128 = 2*cl + c2 (c2 = fb//4, m = fb%4), within-chunk col 32*m + q.
# stg rows: 0-3 x rows (row j feeds chunk j cols), 4-31 transpose window
# zeros, 32-41 static bn/bnc, 42 ones, 43 mx_i, 44 mc_i, 45 t_i.
# SRC staging [32, 516]: x for (j, m) at col 129*j + 32*m; blockwise
# transpose SRC -> stg[0:32] puts x at stg[j, 128*j + 32*m + q].

N_WARM = 0    # PE p-state warmer matmuls per stream-step


def build(n_steps=N_STEPS):
    nc = bacc.Bacc("TRN2", target_bir_lowering=False, debug=False)

    def din(name, shape, dtype):
        return nc.dram_tensor(name, list(shape), dtype, kind="ExternalInput").ap()

    STATIC11 = din("static11", (11, B), F16)      # 10 features + ones row
    MXMCT = din("mxmct", (n_steps, 3, B), F16)    # mx, mc, t rows
    D_ALL = din("d_all", (2, 32, 16 * n_steps), F32)
    B2EFF = din("b2eff", (128, 1), F32)
    W1A_D = din("w1a", (46, 128), F16)
    W2S_D = din("w2s", (128, 128), F16)
    W3S_D = din("w3s", (128, 2), F16)
    SCAL = din("scal", (32, 2), F32)
    SRCINIT = din("srcinit", (32, 516), F16)
    XH0 = din("xh0", (64, 16), F32)

    OUT = nc.dram_tensor("out", [64, 16 * (n_steps + 1)], F32,
                         kind="ExternalOutput").ap()

    with tile.TileContext(nc) as tc:
        import contextlib

        with contextlib.ExitStack() as ctx:
            const = ctx.enter_context(tc.tile_pool(name="const", bufs=1))
            stgp = ctx.enter_context(tc.tile_pool(name="stg", bufs=1))
            h1p = ctx.enter_context(tc.tile_pool(name="h1", bufs=2))
            h2p = ctx.enter_context(tc.tile_pool(name="h2", bufs=2))
            updp = ctx.enter_context(tc.tile_pool(name="upd", bufs=2))
            ps1 = ctx.enter_context(tc.tile_pool(name="ps1", bufs=2, space="PSUM"))
            ps2 = ctx.enter_context(tc.tile_pool(name="ps2", bufs=1, space="PSUM"))
            ps3 = ctx.enter_context(tc.tile_pool(name="ps3", bufs=1, space="PSUM"))

            w1a = const.tile([46, 128], F16)
            nc.sync.dma_start(w1a[:], W1A_D)
            w2r = const.tile([128, 128], F16)
            nc.sync.dma_start(w2r[:], W2S_D)
            w3b = const.tile([128, 2], F16)
            nc.sync.dma_start(w3b[:], W3S_D)
            b2eff = const.tile([128, 1], F32)
            nc.sync.dma_start(b2eff[:], B2EFF)
            d_all = [const.tile([32, 16 * n_steps], F32, tag=f"dall{s}",
                                name=f"dall{s}") for s in range(2)]
            for s in range(2):
                nc.sync.dma_start(d_all[s][:], D_ALL[s])
            scal = const.tile([32, 2], F32)
            nc.sync.dma_start(scal[:], SCAL)
            b3s = scal[:, 0:1]
            bc3s = scal[:, 1:2]

            xh_hist = [const.tile([32, 16 * (n_steps + 1)], F32, tag=f"xh{s}",
                                  name=f"xh{s}") for s in range(2)]
            for s in range(2):
                nc.sync.dma_start(xh_hist[s][:, 0:16], XH0[32 * s:32 * s + 32])

            src = [const.tile([32, 516], F16, tag=f"src{s}", name=f"src{s}")
                   for s in range(2)]
            for s in range(2):
                nc.sync.dma_start(src[s][:], SRCINIT)

            def src_v(t):
                # [32, cl, c2, m] view at col 129*(2*cl+c2) + 32*m
                v = t[:].rearrange("p (j r) -> p j r", j=4)
                v = v[:, :, 0:128].rearrange("p j (m t) -> p j m t", m=4)
                return v[:, :, :, 0].rearrange("p (cl c2) m -> p cl c2 m", cl=2)

            p3sp = [ps3.tile([32, 512], F32, tag=f"p3sp{s}", name=f"p3sp{s}")
                    for s in range(2)]
            for s in range(2):
                nc.vector.memset(p3sp[s][:], 0.0)

            if N_WARM:
                warm_w = const.tile([8, 8], F16)
                nc.vector.memset(warm_w[:], 0.0)
                warm_r = const.tile([8, 64], F16)
                nc.vector.memset(warm_r[:], 0.0)
                warm_ps = ps2.tile([8, 64], F32, name="warmps")

            stg = [stgp.tile([46, B], F16, tag=f"stgt{k}", name=f"stgt{k}")
                   for k in range(3)]
            for k in range(3):
                nc.sync.dma_start(stg[k][32:43, :], STATIC11)
            nc.sync.dma_start(stg[0][43:46, :], MXMCT[0])
            if n_steps > 1:
                nc.sync.dma_start(stg[1][43:46, :], MXMCT[1])
            for s in range(2):
                nc.vector.transpose(stg[0][0:32, 512 * s:512 * (s + 1)],
                                    src[s][:, 0:512])

            skew_anchor = [None]
            h2_live = {}
            src_ins = {}

            h1_live = {}

            def emit_head1(s, i):
                # L1 -> relu1 for stream s, step i
                st = stg[i % 3]
                p1 = ps1.tile([128, SB], F32, tag=f"p1{s}", name=f"p1_{s}_{i}")
                mm1 = nc.tensor.matmul(p1[:], w1a[:],
                                       st[:, 512 * s:512 * (s + 1)],
                                       start=True, stop=True)
                if i == 0 and s == 1 and skew_anchor[0] is not None:
                    add_dep_helper(mm1.ins, skew_anchor[0], sync=True,
                                   reason="stream anti-phase skew")
                h1 = h1p.tile([128, SB], F16, tag=f"h1{s}", name=f"h1_{s}_{i}")
                nc.scalar.activation(h1[:], p1[:], ACTF.Relu)
                h1_live[s] = h1

            def emit_head2(s, i):
                # L2 -> relu2 for stream s, step i
                p2 = ps2.tile([128, SB], F32, tag=f"p2{s}", name=f"p2_{s}_{i}")
                nc.tensor.matmul(p2[:], w2r[:], h1_live[s][:],
                                 start=True, stop=True)
                h2 = h2p.tile([128, SB], F16, tag=f"h2{s}", name=f"h2_{s}_{i}")
                nc.scalar.activation(h2[:], p2[:], ACTF.Relu, bias=b2eff[:])
                h2_live[s] = h2

            def emit_tail(s, i):
                # L3 -> p3t -> update -> src -> stgT for stream s, step i
                nc.tensor.matmul(p3sp[s][0:2, :], w3b[:], h2_live[s][:],
                                 start=True, stop=True)
                p3t = updp.tile([32, 512], F32, tag=f"p3t{s}",
                                name=f"p3t_{s}_{i}")
                tr3 = nc.vector.transpose(p3t[:], p3sp[s][:])
                if i == 0 and s == 0:
                    skew_anchor[0] = tr3.ins
                # serialize the two streams' VE tails: each p3t waits for the
                # other stream's srcmul so tails run as uninterrupted blocks
                other = src_ins.get(1 - s)
                if other is not None:
                    add_dep_helper(tr3.ins, other, sync=True,
                                   reason="VE tail anti-phase")
                p3t_r = p3t[:].rearrange("p (c r) -> p c r", r=32)
                pi_view = p3t_r[:, :, 0]
                lc_view = p3t_r[:, :, 1]
                cdt = updp.tile([32, 16], F32, tag=f"cdt{s}",
                                name=f"cdt_{s}_{i}")
                nc.scalar.activation(cdt[:], lc_view, ACTF.Exp, bias=bc3s)
                xprev = xh_hist[s][:, 16 * i:16 * (i + 1)]
                xnext = xh_hist[s][:, 16 * (i + 1):16 * (i + 2)]
                xprev_r = xprev.rearrange("p (cl c2 m) -> p cl c2 m",
                                          cl=2, c2=2)
                a_t = updp.tile([32, 16], F32, tag=f"a{s}", name=f"a_{s}_{i}")
                g_t = updp.tile([32, 16], F32, tag=f"g{s}", name=f"g_{s}_{i}")
                g_r = g_t[:].rearrange("p (cl c2 m) -> p cl c2 m", cl=2, c2=2)
                with tc.high_priority():
                    nc.vector.scalar_tensor_tensor(
                        a_t[:], pi_view, b3s,
                        d_all[s][:, 16 * i:16 * (i + 1)], ALU.add, ALU.mult)
                    nc.vector.scalar_tensor_tensor(
                        g_t[:], a_t[:], 1.0, cdt[:], ALU.add, ALU.subtract)
                    sm = nc.vector.tensor_mul(src_v(src[s]), xprev_r, g_r)
                src_ins[s] = sm.ins
                if i + 1 < n_steps:
                    nc.vector.transpose(
                        stg[(i + 1) % 3][0:32, 512 * s:512 * (s + 1)],
                        src[s][:, 0:512])
                nc.gpsimd.tensor_mul(xnext, xprev, g_t[:])

            # Software-pipelined emission: stream B's tail is emitted one
            # iteration late so each engine's in-order queue matches the
            # anti-phase firing order (B's tail fires early in the next
            # period, before A's MLP ops of that step complete).
            for i in range(n_steps):
                if i + 2 < n_steps:
                    nc.sync.dma_start(stg[(i + 2) % 3][43:46, :], MXMCT[i + 2])
                emit_head1(0, i)
                if i > 0:
                    emit_tail(1, i - 1)
                emit_head2(0, i)
                emit_tail(0, i)
                emit_head1(1, i)
                emit_head2(1, i)
            emit_tail(1, n_steps - 1)

            for s in range(2):
                nc.sync.dma_start(OUT[32 * s:32 * s + 32, :], xh_hist[s][:])

    nc.compile()
    return nc


def host_prep(inputs, n_steps=N_STEPS):
    F16_NP = np.float16
    bm = np.asarray(inputs["bm"], np.float32)
    cn = np.asarray(inputs["cn"], np.float32)
    typeVec = np.asarray(inputs["typeVec"], np.float32)
    mx = np.asarray(inputs["mx"], np.float32)
    mc = np.asarray(inputs["mc"], np.float32)
    initial = float(np.asarray(inputs["initial"]).reshape(-1)[0])
    bn_gamma = np.asarray(inputs["bn_gamma"], np.float32)
    bn_beta = np.asarray(inputs["bn_beta"], np.float32)
    bnc_gamma = np.asarray(inputs["bnc_gamma"], np.float32)
    bnc_beta = np.asarray(inputs["bnc_beta"], np.float32)
    w1 = np.asarray(inputs["w1"], np.float32)
    b1 = np.asarray(inputs["b1"], np.float32)
    w2 = np.asarray(inputs["w2"], np.float32)
    b2 = np.asarray(inputs["b2"], np.float32)
    w3 = np.asarray(inputs["w3"], np.float32)
    b3 = np.asarray(inputs["b3"], np.float32)
    wc1 = np.asarray(inputs["wc1"], np.float32)
    bc1 = np.asarray(inputs["bc1"], np.float32)
    wc2 = np.asarray(inputs["wc2"], np.float32)
    bc2 = np.asarray(inputs["bc2"], np.float32)
    wc3 = np.asarray(inputs["wc3"], np.float32)
    bc3 = np.asarray(inputs["bc3"], np.float32)

    Bg, N, _ = bm.shape
    assert Bg == B_GLOBAL and N >= n_steps

    m = typeVec.mean(axis=0, dtype=np.float64)
    v = ((typeVec.astype(np.float64) - m) ** 2).mean(axis=0)
    inv = 1.0 / np.sqrt(v + BN_EPS)
    bn = ((typeVec - m) * inv * bn_gamma + bn_beta).astype(np.float32)
    bnc = ((typeVec - m) * inv * bnc_gamma + bnc_beta).astype(np.float32)

    dcn = cn[:, 1:n_steps + 1, 0] - cn[:, :n_steps, 0]
    drift = (np.float32(MU * DT) + np.float32(NU) * bm[:, :n_steps, 0]
             + np.float32(SIGMA) * dcn).astype(np.float32)
    mxs = mx[:, :n_steps, 0]
    mcs = mc[:, :n_steps, 0]
    ts = (np.arange(n_steps, dtype=np.float32) * np.float32(DT))

    def stack_row(a, b_):
        return np.concatenate([a, b_]).astype(np.float32)

    w1a = np.zeros((46, 128), np.float32)
    wx = stack_row(w1[6], wc1[6])
    for j in range(4):
        w1a[j] = wx
    for k in range(5):
        w1a[32 + k, 0:64] = w1[k]
        w1a[37 + k, 64:128] = wc1[k]
    w1a[42] = stack_row(b1, bc1)          # ones row -> layer-1 bias
    w1a[43] = stack_row(w1[7], wc1[7])    # mx
    w1a[44] = stack_row(w1[8], wc1[8])    # mc
    w1a[45] = stack_row(w1[5], wc1[5])    # t

    b2eff = np.concatenate([b2, bc2]).astype(np.float32).reshape(128, 1)
    w2s = np.zeros((128, 128), np.float32)
    w2s[0:64, 0:64] = w2
    w2s[64:128, 64:128] = wc2
    w3s = np.zeros((128, 2), np.float32)
    w3s[0:64, 0] = w3[:, 0]
    w3s[64:128, 1] = wc3[:, 0]
    scal = np.zeros((32, 2), np.float32)
    scal[:, 0] = b3[0]
    scal[:, 1] = np.float32(bc3[0] + np.log(DT))

    # per-stream index maps: bs = 32*c + q  (c = 8*cl + fb)
    bs = np.arange(SB)
    c_i, q_i = bs // 32, bs % 32

    srcinit = np.zeros((32, 516), np.float32)
    for j in range(4):
        for mm_ in range(4):
            srcinit[:, 129 * j + 32 * mm_] = initial
    xh0 = np.full((64, 16), initial, np.float32)

    in_maps = []
    for core in range(N_CORES):
        sl = slice(core * B, (core + 1) * B)
        bn_c, bnc_c = bn[sl], bnc[sl]
        static11 = np.empty((11, B), np.float32)
        static11[0:5] = bn_c.T
        static11[5:10] = bnc_c.T
        static11[10] = 1.0
        mxmct = np.empty((n_steps, 3, B), np.float32)
        mxmct[:, 0, :] = mxs[sl].T
        mxmct[:, 1, :] = mcs[sl].T
        mxmct[:, 2, :] = ts[:, None]
        d_np = np.zeros((2, 32, 16 * n_steps), np.float32)
        dr = drift[sl]
        for s in range(2):
            drs = dr[512 * s:512 * (s + 1)]    # (512, n_steps)
            d_np[s][q_i[:, None], 16 * np.arange(n_steps)[None, :] + c_i[:, None]] = drs
        in_maps.append({
            "static11": static11.astype(F16_NP),
            "mxmct": mxmct.astype(F16_NP),
            "d_all": d_np,
            "b2eff": b2eff.copy(),
            "w1a": w1a.astype(F16_NP),
            "w2s": w2s.astype(F16_NP),
            "w3s": w3s.astype(F16_NP),
            "scal": scal.copy(),
            "srcinit": srcinit.astype(F16_NP),
            "xh0": xh0.copy(),
        })
    decode = (q_i, c_i)
    return in_maps, decode


def assemble_output(results, decode, n_steps=N_STEPS):
    q_i, c_i = decode
    states = np.empty((B_GLOBAL, n_steps + 1), np.float32)
    cols = 16 * np.arange(n_steps + 1)[None, :] + c_i[:, None]
    for core in range(N_CORES):
        out = results[core]["out"]              # (64, 16*(n_steps+1))
        for s in range(2):
            rows = 32 * s + q_i
            states[core * B + 512 * s:core * B + 512 * (s + 1)] = out[rows[:, None], cols]
    times = (np.arange(n_steps + 1, dtype=np.float32) * np.float32(DT))
    full = np.empty((B_GLOBAL, n_steps + 1, 2), np.float32)
    full[:, :, 0] = times[None, :]
    full[:, :, 1] = states
    return full


_BUILT = {}


def _get_built(n_steps=N_STEPS):
    if n_steps not in _BUILT:
        _BUILT[n_steps] = build(n_steps)
    return _BUILT[n_steps]


def kernel(**inputs):
    nc = _get_built()
    in_maps, decode = host_prep(inputs)
    res = run_bass_kernel_spmd(nc, in_maps, core_ids=list(range(N_CORES)))
    return assemble_output(res.results, decode)


if __name__ == "__main__":
    sys.path.insert(0, os.path.dirname(os.path.abspath(__file__)))
    import reference

    inputs = reference.setup_inputs()
    inputs = {k: np.asarray(v) for k, v in inputs.items()}
    expected = np.asarray(reference.reference(**inputs))
    actual = kernel(**inputs)
    err = np.abs(actual - expected)
    print("max abs err:", err.max())
    print("rel err (scale):", err.max() / np.abs(expected).max())

